# revision 9
# baseline (speedup 1.0000x reference)
"""Trainium2 Bass kernel for nn_BasicBlock_8057358647809 (CDConv message passing).

kernel(**inputs) takes the FULL unsharded inputs (as in reference.setup_inputs())
and returns the FULL [N, 128] output. Edges are partitioned across 8 NeuronCores
by destination-node windows of 128 nodes (49 windows/core); the input MLP is
computed distributed (1/8 of nodes per core) with AllReduced BN stats and an
AllGathered h table; messages use bf16 for the outer-product/aggregation path.

Host-side work is layout only: sorting edges by destination, packing per-edge
index/feature streams, padding. All arithmetic from the reference runs on
device.
"""

import math
import os
import sys
from dataclasses import dataclass

for _p in ("/root/.axon_site/_ro/trn_rl_repo", "/opt/trn_rl_repo"):
    if os.path.isdir(_p) and _p not in sys.path:
        sys.path.append(_p)

import numpy as np
import ml_dtypes
import concourse.bass as bass
import concourse.bacc as bacc
import concourse.mybir as mybir
import concourse.tile as tile
from concourse import bass_utils

F32 = mybir.dt.float32
BF16 = mybir.dt.bfloat16
I32 = mybir.dt.int32
AF = mybir.ActivationFunctionType
ALU = mybir.AluOpType
AX = mybir.AxisListType
P = 128
NPBF = np.dtype(ml_dtypes.bfloat16)

BN_EPS = 1e-5
RADIUS = 4.0
SHALF = 5.0  # L // 2


@dataclass(frozen=True)
class Cfg:
    n_real: int = 50000      # real node count (BN divisor)
    nc: int = 8              # cores
    nwc: int = 49            # node windows (of 128) per core
    K: int = 9               # edge tiles per window (host-computed)
    sim: bool = False

    @property
    def npad(self):
        return self.nc * self.nwc * P

    @property
    def npc(self):
        return self.nwc * P

    @property
    def T(self):
        return self.nwc * self.K


# ---------------------------------------------------------------- host prep

def prep_host(cfg: Cfg, x, pos, seq, ori, edge_index,
              Wn0, bn0, W_in, Wc, W_out,
              g_in, b_in, g_mid, b_mid, g_out, b_out):
    N = x.shape[0]
    E = edge_index.shape[1]
    npad = cfg.npad

    xpad = np.zeros((npad, 128), np.float32)
    xpad[:N] = x

    atab = np.zeros((npad, 16), np.float32)
    atab[:N, 0:3] = pos
    atab[:N, 3] = seq[:, 0]
    atab[:N, 4:13] = ori.reshape(N, 9)

    src = edge_index[0].astype(np.int64)
    dst = edge_index[1].astype(np.int64)
    order = np.argsort(dst, kind="stable")
    src_s = src[order]
    dst_s = dst[order]

    nwin = cfg.nc * cfg.nwc
    w_of_edge = dst_s // P
    counts = np.bincount(w_of_edge, minlength=nwin)
    K = max(1, int(math.ceil(counts.max() / P)))
    if K != cfg.K:
        cfg = Cfg(cfg.n_real, cfg.nc, cfg.nwc, K, cfg.sim)
    T = cfg.T

    src_arena = np.zeros((cfg.nc, P, T), np.int64)
    dst_arena = np.zeros((cfg.nc, P, T), np.int64)
    dloc_arena = np.full((cfg.nc, P, T), -1.0, np.float32)

    starts = np.zeros(nwin + 1, np.int64)
    np.cumsum(counts, out=starts[1:])
    pos_in_w = np.arange(E) - starts[w_of_edge]
    core = w_of_edge // cfg.nwc
    wl = w_of_edge % cfg.nwc
    t_local = wl * K + pos_in_w // P
    prow = pos_in_w % P
    src_arena[core, prow, t_local] = src_s
    dst_arena[core, prow, t_local] = dst_s
    dloc_arena[core, prow, t_local] = (dst_s - w_of_edge * P).astype(np.float32)

    # WeightNet stacked table [(l,j), k]: j==7 is the bias row; padded to 128
    wstack = np.zeros((128, 16), np.float32)
    for l in range(11):
        wstack[l * 8:l * 8 + 7, :] = Wn0[l]
        wstack[l * 8 + 7, :] = bn0[l]

    iota128 = np.broadcast_to(np.arange(P, dtype=np.float32), (P, P)).copy()
    iota11m5 = np.broadcast_to(np.arange(11, dtype=np.float32) - SHALF,
                               (P, 11)).copy()
    ident = np.eye(P, dtype=np.float32)

    wc_packed = np.zeros((P, 128), np.float32)
    for c in range(4):
        wc_packed[:, c * 32:(c + 1) * 32] = Wc[c * 128:(c + 1) * 128, :]

    common = dict(
        wstackb=wstack.astype(NPBF), wcb=wc_packed.astype(NPBF),
        win=W_in.astype(np.float32), wout=W_out.astype(np.float32),
        g_in=g_in.reshape(128, 1).astype(np.float32),
        b_in=b_in.reshape(128, 1).astype(np.float32),
        g_mid=g_mid.reshape(32, 1).astype(np.float32),
        b_mid=b_mid.reshape(32, 1).astype(np.float32),
        g_out=g_out.reshape(32, 1).astype(np.float32),
        b_out=b_out.reshape(32, 1).astype(np.float32),
        iota128=iota128, iota11m5=iota11m5, ident=ident,
        identb=ident.astype(NPBF),
        ones1=np.ones((P, 1), np.float32),
        m14=np.full((P, 1), -14.0, np.float32),
    )
    common = {k: np.ascontiguousarray(v) for k, v in common.items()}

    in_maps = []
    for c in range(cfg.nc):
        m = dict(common)
        m["x_slice"] = np.ascontiguousarray(
            xpad[c * cfg.npc:(c + 1) * cfg.npc])
        m["asrcf"] = np.ascontiguousarray(
            atab[src_arena[c]].reshape(P, T * 16))
        m["adstf"] = np.ascontiguousarray(
            atab[dst_arena[c]].reshape(P, T * 16))
        m["srcidx"] = np.ascontiguousarray(src_arena[c].astype(np.int32))
        m["dloc"] = np.ascontiguousarray(dloc_arena[c])
        in_maps.append(m)
    return cfg, in_maps


# ---------------------------------------------------------------- builder

def build_nc(cfg: Cfg):
    nc = bacc.Bacc("TRN2", target_bir_lowering=False, debug=False,
                   enable_asserts=False, num_devices=cfg.nc)

    npad = cfg.npad
    npc = cfg.npc
    T = cfg.T
    K = cfg.K
    NW = cfg.nwc
    inv_n = 1.0 / cfg.n_real

    def din(name, shape, dt_=F32):
        return nc.dram_tensor(name, shape, dt_, kind="ExternalInput")

    x_slice = din("x_slice", [npc, 128])
    asrcf = din("asrcf", [P, T * 16])
    adstf = din("adstf", [P, T * 16])
    srcidx = din("srcidx", [P, T], I32)
    dloc = din("dloc", [P, T])
    win = din("win", [128, 32])
    wout = din("wout", [32, 128])
    wstackb = din("wstackb", [128, 16], BF16)
    wcb = din("wcb", [128, 128], BF16)
    g_in = din("g_in", [128, 1]); b_in = din("b_in", [128, 1])
    g_mid = din("g_mid", [32, 1]); b_mid = din("b_mid", [32, 1])
    g_out = din("g_out", [32, 1]); b_out = din("b_out", [32, 1])
    iota128 = din("iota128", [P, P])
    iota11m5 = din("iota11m5", [P, 11])
    ident_d = din("ident", [P, P])
    identb_d = din("identb", [P, P], BF16)
    ones1_d = din("ones1", [P, 1])
    m14_d = din("m14", [P, 1])

    out_slice = nc.dram_tensor("out_slice", [npc, 128], F32,
                               kind="ExternalOutput")

    def av(base_ap, off, pat):
        """Arena view: AP at element offset `off` with free pattern `pat`."""
        b = base_ap[:, off:off + 1] if off else base_ap[:, 0:1]
        return bass.AP(b.tensor, b.offset, [b.ap[0]] + [list(p) for p in pat])

    with tile.TileContext(nc) as tc:
        import contextlib
        ctx = contextlib.ExitStack()
        with ctx:
            cpool = ctx.enter_context(tc.tile_pool(name="consts", bufs=1))
            apool = ctx.enter_context(tc.tile_pool(name="arena", bufs=1))
            spool = ctx.enter_context(tc.tile_pool(name="stats", bufs=1))
            wkp = ctx.enter_context(
                tc.tile_pool(name="psw", bufs=3, space="PSUM"))
            dram = ctx.enter_context(
                tc.tile_pool(name="dram", bufs=1, space="DRAM"))
            mpool_cm = tc.tile_pool(name="mid", bufs=1)
            mpool = mpool_cm.__enter__()
            gpool_cm = tc.tile_pool(name="geo", bufs=1)
            gpool = gpool_cm.__enter__()

            def cload(dram_t, shape, tag, dt_=F32):
                t = cpool.tile(shape, dt_, tag=tag)
                nc.sync.dma_start(t[:], dram_t[:])
                return t

            ident_s = cload(ident_d, [P, P], "ident")
            identb_s = cload(identb_d, [P, P], "identb", BF16)
            iota128_s = cload(iota128, [P, P], "iota128")
            iota11_s = cload(iota11m5, [P, 11], "iota11")
            ones1_s = cload(ones1_d, [P, 1], "ones1")
            m14_s = cload(m14_d, [P, 1], "m14")
            wstack_s = cload(wstackb, [128, 16], "wstack", BF16)
            win_s = cload(win, [128, 32], "win")
            wcb_s = cload(wcb, [128, 128], "wcb", BF16)
            wout_s = cload(wout, [32, 128], "wout")
            g_in_s = cload(g_in, [128, 1], "g_in")
            b_in_s = cload(b_in, [128, 1], "b_in")
            g_mid_s = cload(g_mid, [32, 1], "g_mid")
            b_mid_s = cload(b_mid, [32, 1], "b_mid")
            g_out_s = cload(g_out, [32, 1], "g_out")
            b_out_s = cload(b_out, [32, 1], "b_out")
            srcidx_s = cload(srcidx, [P, T], "srcidx", I32)
            dloc_s = cload(dloc, [P, T], "dloc")
            asrc_s = gpool.tile([P, T * 16], F32, tag="asrc")
            nc.sync.dma_start(asrc_s[:], asrcf[:])
            adst_s = gpool.tile([P, T * 16], F32, tag="adst")
            nc.sync.dma_start(adst_s[:], adstf[:])

            htabp = dram.tile([npad, 16], F32)   # h table (bf16 packed)
            hloc = dram.tile([npc, 16], F32)

            def bn_coefs(sum_ap, sumsq_ap, g_ap, b_ap, n_part, pfx):
                mean = spool.tile([n_part, 1], F32, tag=pfx + "mean")
                nc.vector.tensor_scalar_mul(mean[:], sum_ap, inv_n)
                var = spool.tile([n_part, 1], F32, tag=pfx + "var")
                nc.vector.tensor_scalar_mul(var[:], sumsq_ap, inv_n)
                m2 = spool.tile([n_part, 1], F32, tag=pfx + "m2")
                nc.vector.tensor_tensor(out=m2[:], in0=mean[:], in1=mean[:],
                                        op=ALU.mult)
                nc.vector.tensor_tensor(out=var[:], in0=var[:], in1=m2[:],
                                        op=ALU.subtract)
                nc.vector.tensor_scalar_add(var[:], var[:], BN_EPS)
                sd = spool.tile([n_part, 1], F32, tag=pfx + "sd")
                nc.scalar.activation(out=sd[:], in_=var[:], func=AF.Sqrt)
                rstd = spool.tile([n_part, 1], F32, tag=pfx + "rstd")
                nc.vector.reciprocal(rstd[:], sd[:])
                A = spool.tile([n_part, 1], F32, tag=pfx + "A")
                nc.vector.tensor_tensor(out=A[:], in0=g_ap, in1=rstd[:],
                                        op=ALU.mult)
                B = spool.tile([n_part, 1], F32, tag=pfx + "B")
                nc.vector.tensor_tensor(out=B[:], in0=mean[:], in1=A[:],
                                        op=ALU.mult)
                nc.vector.tensor_tensor(out=B[:], in0=b_ap, in1=B[:],
                                        op=ALU.subtract)
                return A, B

            def allreduce(src_tile, shape, tag):
                ib = dram.tile(shape, F32, tag=tag + "_i")
                ob = dram.tile(shape, F32, tag=tag + "_o")
                nc.gpsimd.dma_start(ib[:], src_tile[:])
                if cfg.nc > 1:
                    nc.gpsimd.collective_compute(
                        "AllReduce", ALU.add,
                        replica_groups=[list(range(cfg.nc))],
                        ins=[ib.opt()], outs=[ob.opt()])
                else:
                    nc.gpsimd.dma_start(ob[:], ib[:])
                r = spool.tile(shape, F32, tag=tag + "_r")
                nc.sync.dma_start(r[:], ob[:])
                return r

            def leaky_dve(out_ap, in_ap, pool, tag, shape):
                tmp = pool.tile(shape, F32, tag=tag)
                nc.vector.tensor_scalar_mul(tmp[:], in_ap, 0.1)
                nc.vector.tensor_tensor(out=out_ap, in0=in_ap, in1=tmp[:],
                                        op=ALU.max)

            # ============ PHASE A: local input-BN stats
            with tc.tile_pool(name="psA", bufs=1, space="PSUM") as psA, \
                 tc.tile_pool(name="pA", bufs=4) as pA:
                S12 = psA.tile([128, 129], F32, tag="S12")
                for i in range(NW):
                    x_t = pA.tile([P, 129], F32, tag="xa")
                    nc.sync.dma_start(x_t[:, 0:128],
                                      x_slice[i * P:(i + 1) * P, :])
                    nc.vector.tensor_copy(out=x_t[:, 128:129], in_=ones1_s[:])
                    nc.tensor.matmul(out=S12[:], lhsT=x_t[:, 0:128],
                                     rhs=x_t[:], start=(i == 0),
                                     stop=(i == NW - 1),
                                     skip_group_check=True)
                s2m = spool.tile([128, 128], F32, tag="s2m")
                nc.vector.tensor_tensor(out=s2m[:], in0=S12[:, 0:128],
                                        in1=ident_s[:], op=ALU.mult)
                cc1 = spool.tile([128, 2], F32, tag="cc1")
                nc.vector.tensor_copy(out=cc1[:, 0:1], in_=S12[:, 128:129])
                nc.vector.tensor_reduce(out=cc1[:, 1:2], in_=s2m[:],
                                        axis=AX.X, op=ALU.add)
            cc1r = allreduce(cc1, [128, 2], "ar1")
            A1, B1 = bn_coefs(cc1r[:, 0:1], cc1r[:, 1:2], g_in_s[:],
                              b_in_s[:], 128, "a_")

            # ============ PHASE B: local h0 + mid-BN stats
            h0pool_cm = tc.tile_pool(name="h0p", bufs=1)
            h0pool = h0pool_cm.__enter__()
            h0_arena = h0pool.tile([P, NW * 33], F32, tag="h0ar")
            with tc.tile_pool(name="psB", bufs=1, space="PSUM") as psB, \
                 tc.tile_pool(name="pB", bufs=4) as pB:
                Sh = psB.tile([33, 33], F32, tag="Sh")
                for i in range(NW):
                    x_t = pB.tile([P, 128], F32, tag="xb")
                    nc.sync.dma_start(x_t[:], x_slice[i * P:(i + 1) * P, :])
                    xT = wkp.tile([P, 128], F32, tag="wk")
                    nc.tensor.transpose(out=xT[:], in_=x_t[:],
                                        identity=ident_s[:])
                    t_cn = pB.tile([P, 128], F32, tag="tcn")
                    nc.scalar.activation(out=t_cn[:], in_=xT[:],
                                         func=AF.Identity,
                                         bias=B1[:, 0:1], scale=A1[:, 0:1])
                    leaky_dve(t_cn[:], t_cn[:], pB, "lkB", [P, 128])
                    h0p = wkp.tile([P, 32], F32, tag="wk")
                    nc.tensor.matmul(out=h0p[:], lhsT=t_cn[:], rhs=win_s[:],
                                     start=True, stop=True)
                    h0v = h0_arena[:, i * 33:i * 33 + 32]
                    nc.vector.tensor_copy(out=h0v, in_=h0p[:])
                    nc.vector.tensor_copy(
                        out=h0_arena[:, i * 33 + 32:i * 33 + 33],
                        in_=ones1_s[:])
                    nc.tensor.matmul(out=Sh[:],
                                     lhsT=h0_arena[:, i * 33:(i + 1) * 33],
                                     rhs=h0_arena[:, i * 33:(i + 1) * 33],
                                     start=(i == 0), stop=(i == NW - 1),
                                     skip_group_check=True)
                shs = spool.tile([33, 33], F32, tag="shs")
                nc.vector.tensor_copy(out=shs[:], in_=Sh[:])
                shm = spool.tile([32, 32], F32, tag="shm")
                nc.vector.tensor_tensor(out=shm[:], in0=shs[0:32, 0:32],
                                        in1=ident_s[0:32, 0:32], op=ALU.mult)
                cc2 = spool.tile([32, 2], F32, tag="cc2")
                nc.vector.tensor_copy(out=cc2[:, 0:1], in_=shs[0:32, 32:33])
                nc.vector.tensor_reduce(out=cc2[:, 1:2], in_=shm[:],
                                        axis=AX.X, op=ALU.add)
            cc2r = allreduce(cc2, [32, 2], "ar2")
            A2, B2 = bn_coefs(cc2r[:, 0:1], cc2r[:, 1:2], g_mid_s[:],
                              b_mid_s[:], 32, "b_")

            # ============ PHASE C: local h (bf16) -> hloc; AllGather
            with tc.tile_pool(name="pC", bufs=4) as pC:
                for i in range(NW):
                    h0v = h0_arena[:, i * 33:i * 33 + 32]
                    hTp = wkp.tile([32, 128], F32, tag="wk")
                    nc.tensor.transpose(out=hTp[:], in_=h0v,
                                        identity=ident_s[:])
                    hT = pC.tile([32, 128], F32, tag="hT")
                    nc.scalar.activation(out=hT[:], in_=hTp[:],
                                         func=AF.Identity,
                                         bias=B2[:, 0:1], scale=A2[:, 0:1])
                    leaky_dve(hT[:], hT[:], pC, "lkC", [32, 128])
                    hbp = wkp.tile([P, 32], F32, tag="wk")
                    nc.tensor.transpose(out=hbp[:], in_=hT[:],
                                        identity=ident_s[0:32, 0:32])
                    h_t = pC.tile([P, 32], BF16, tag="h_t")
                    nc.vector.tensor_copy(out=h_t[:], in_=hbp[:])
                    nc.sync.dma_start(hloc[i * P:(i + 1) * P, :],
                                      h_t[:].bitcast(F32))
            h0pool_cm.__exit__(None, None, None)
            if cfg.nc > 1:
                nc.gpsimd.collective_compute(
                    "AllGather", ALU.bypass,
                    replica_groups=[list(range(cfg.nc))],
                    ins=[hloc.opt()], outs=[htabp.opt()])
            else:
                nc.gpsimd.dma_start(htabp[:], hloc[:])

            # ============ PHASE D1: edge geometry (arena-wide)
            sc3 = [16, T]
            rel_a = gpool.tile([P, 3 * T], F32, tag="rel")
            big9 = gpool.tile([P, 9 * T], F32, tag="big9")
            dirn_a = gpool.tile([P, 3 * T], F32, tag="dirn")
            delta_a = mpool.tile([P, 8 * T], F32, tag="delta")
            S_a = mpool.tile([P, 11 * T], F32, tag="S_a")
            d2_a = gpool.tile([P, T], F32, tag="d2")
            inv_a = gpool.tile([P, T], F32, tag="inv")
            seqd_a = gpool.tile([P, T], F32, tag="seqd")
            absd_a = gpool.tile([P, T], F32, tag="absd")
            sm_a = mpool.tile([P, T], F32, tag="sm")
            kws_a = apool.tile([P, 16 * T], BF16, tag="kws")

            V = nc.vector
            V.tensor_tensor(out=av(rel_a[:], 0, [[3, T], [1, 3]]),
                            in0=av(asrc_s[:], 0, [sc3, [1, 3]]),
                            in1=av(adst_s[:], 0, [sc3, [1, 3]]),
                            op=ALU.subtract)
            V.tensor_tensor(out=av(big9[:], 0, [[3, T], [1, 3]]),
                            in0=rel_a[:, 0:3 * T], in1=rel_a[:, 0:3 * T],
                            op=ALU.mult)
            V.tensor_reduce(out=d2_a[:],
                            in_=av(big9[:], 0, [[3, T], [1, 3]]),
                            axis=AX.X, op=ALU.add)
            nc.scalar.activation(out=av(delta_a[:], 6, [[8, T], [1, 1]]),
                                 in_=d2_a[:], func=AF.Sqrt)
            V.tensor_scalar_add(inv_a[:],
                                av(delta_a[:], 6, [[8, T], [1, 1]]), 1e-9)
            V.reciprocal(inv_a[:], inv_a[:])
            V.tensor_tensor(out=av(dirn_a[:], 0, [[3, T], [1, 3]]),
                            in0=av(rel_a[:], 0, [[3, T], [1, 3]]),
                            in1=av(inv_a[:], 0, [[1, T], [0, 3]]),
                            op=ALU.mult)
            V.tensor_tensor(out=av(big9[:], 0, [[9, T], [3, 3], [1, 3]]),
                            in0=av(adst_s[:], 4, [sc3, [3, 3], [1, 3]]),
                            in1=av(dirn_a[:], 0, [[3, T], [0, 3], [1, 3]]),
                            op=ALU.mult)
            V.tensor_reduce(out=av(delta_a[:], 0, [[8, T], [1, 3]]),
                            in_=av(big9[:], 0, [[9, T], [3, 3], [1, 3]]),
                            axis=AX.X, op=ALU.add)
            V.tensor_tensor(out=av(big9[:], 0, [[9, T], [1, 9]]),
                            in0=av(adst_s[:], 4, [sc3, [1, 9]]),
                            in1=av(asrc_s[:], 4, [sc3, [1, 9]]),
                            op=ALU.mult)
            V.tensor_reduce(out=av(delta_a[:], 3, [[8, T], [1, 3]]),
                            in_=av(big9[:], 0, [[9, T], [3, 3], [1, 3]]),
                            axis=AX.X, op=ALU.add)
            V.memset(av(delta_a[:], 7, [[8, T], [1, 1]]), 1.0)
            V.tensor_tensor(out=seqd_a[:],
                            in0=av(asrc_s[:], 3, [sc3, [1, 1]]),
                            in1=av(adst_s[:], 3, [sc3, [1, 1]]),
                            op=ALU.subtract)
            V.tensor_scalar(out=seqd_a[:], in0=seqd_a[:], scalar1=SHALF,
                            scalar2=-SHALF, op0=ALU.min, op1=ALU.max)
            V.tensor_tensor(out=av(S_a[:], 0, [[11, T], [1, 11]]),
                            in0=av(seqd_a[:], 0, [[1, T], [0, 11]]),
                            in1=av(iota11_s[:], 0, [[0, T], [1, 11]]),
                            op=ALU.is_equal)
            nc.scalar.activation(out=absd_a[:], in_=seqd_a[:], func=AF.Abs)
            V.tensor_tensor(out=sm_a[:],
                            in0=av(delta_a[:], 6, [[8, T], [1, 1]]),
                            in1=absd_a[:], op=ALU.mult)
            nc.scalar.activation(out=sm_a[:], in_=sm_a[:], func=AF.Tanh,
                                 bias=m14_s[:, 0:1],
                                 scale=16.0 / (RADIUS * SHALF))
            V.tensor_scalar(out=sm_a[:], in0=sm_a[:], scalar1=-0.5,
                            scalar2=0.5, op0=ALU.mult, op1=ALU.add)

            gpool_cm.__exit__(None, None, None)

            # ============ PHASE D2a: WeightNet -> kws arena (bf16)
            SD_w = mpool.tile([P, K * 128], BF16, tag="SD")
            V.memset(SD_w[:], 0.0)
            with tc.tile_pool(name="pDa", bufs=3) as pDa:
                for w in range(NW):
                    t0 = w * K
                    V.tensor_tensor(
                        out=av(SD_w[:], 0, [[128, K], [8, 11], [1, 8]]),
                        in0=av(S_a[:], t0 * 11, [[11, K], [1, 11], [0, 8]]),
                        in1=av(delta_a[:], t0 * 8, [[8, K], [0, 11], [1, 8]]),
                        op=ALU.mult)
                    kwl_w = pDa.tile([P, K * 16], F32, tag="kwl")
                    for k in range(K):
                        SDTp = wkp.tile([128, 128], BF16, tag="wk")
                        nc.tensor.transpose(
                            out=SDTp[:], in_=SD_w[:, k * 128:(k + 1) * 128],
                            identity=identb_s[:])
                        SDT = pDa.tile([128, 128], BF16, tag="SDT")
                        V.tensor_copy(out=SDT[:], in_=SDTp[:])
                        kw0p = wkp.tile([P, 16], F32, tag="wk")
                        nc.tensor.matmul(out=kw0p[:], lhsT=SDT[:],
                                         rhs=wstack_s[:], start=True,
                                         stop=True)
                        kwt = pDa.tile([P, 16], F32, tag="kwt")
                        V.tensor_scalar_mul(kwt[:], kw0p[:], 0.2)
                        V.tensor_tensor(out=kwl_w[:, k * 16:(k + 1) * 16],
                                        in0=kw0p[:], in1=kwt[:], op=ALU.max)
                    V.tensor_tensor(
                        out=av(kws_a[:], t0 * 16, [[16, K], [1, 16]]),
                        in0=kwl_w[:],
                        in1=av(sm_a[:], t0, [[1, K], [0, 16]]),
                        op=ALU.mult)

            mpool_cm.__exit__(None, None, None)

            # ============ PHASE D2b: gather h, messages, aggregate, conv
            convT = apool.tile([32, NW * 128], F32, tag="convT")
            sc_acc = spool.tile([32, 2], F32, tag="sc_acc")
            with tc.tile_pool(name="psAgg", bufs=2, space="PSUM") as psAgg, \
                 tc.tile_pool(name="psCv", bufs=2, space="PSUM") as psCv, \
                 tc.tile_pool(name="pDb", bufs=3) as pDb:
                for w in range(NW):
                    t0 = w * K
                    hsrc_w = pDb.tile([P, K * 16], F32, tag="hsrc")
                    for k in range(K):
                        nc.gpsimd.indirect_dma_start(
                            out=hsrc_w[:, k * 16:(k + 1) * 16],
                            out_offset=None, in_=htabp[:],
                            in_offset=bass.IndirectOffsetOnAxis(
                                ap=srcidx_s[:, t0 + k:t0 + k + 1], axis=0))
                    msg_w = pDb.tile([P, K * 512], BF16, tag="msg")
                    hb = hsrc_w[:].bitcast(BF16)
                    V.tensor_tensor(
                        out=av(msg_w[:], 0, [[512, K], [1, 512]]),
                        in0=av(kws_a[:], t0 * 16, [[16, K], [1, 16], [0, 32]]),
                        in1=av(hb, 0, [[32, K], [0, 16], [1, 32]]),
                        op=ALU.mult)
                    O_w = pDb.tile([P, K * 128], BF16, tag="O_w")
                    V.tensor_tensor(
                        out=av(O_w[:], 0, [[128, K], [1, 128]]),
                        in0=av(dloc_s[:], t0, [[1, K], [0, 128]]),
                        in1=av(iota128_s[:], 0, [[0, K], [1, 128]]),
                        op=ALU.is_equal)
                    aggP = psAgg.tile([P, 512], F32, tag="agg")
                    for k in range(K):
                        nc.tensor.matmul(
                            out=aggP[:], lhsT=O_w[:, k * 128:(k + 1) * 128],
                            rhs=msg_w[:, k * 512:(k + 1) * 512],
                            start=(k == 0), stop=(k == K - 1),
                            skip_group_check=True)
                    aggB = pDb.tile([P, 512], BF16, tag="aggB")
                    V.tensor_copy(out=aggB[:], in_=aggP[:])
                    convTp = psCv.tile([32, 128], F32, tag="cv")
                    for c in range(4):
                        aTp = wkp.tile([P, 128], BF16, tag="wk")
                        nc.tensor.transpose(
                            out=aTp[:], in_=aggB[:, c * 128:(c + 1) * 128],
                            identity=identb_s[:])
                        aTs = pDb.tile([P, 128], BF16, tag="aTs")
                        V.tensor_copy(out=aTs[:], in_=aTp[:])
                        nc.tensor.matmul(out=convTp[:],
                                         lhsT=wcb_s[:, c * 32:(c + 1) * 32],
                                         rhs=aTs[:], start=(c == 0),
                                         stop=(c == 3),
                                         skip_group_check=True)
                    cv = convT[:, w * 128:(w + 1) * 128]
                    V.tensor_copy(out=cv, in_=convTp[:])
                    sq = pDb.tile([32, 128], F32, tag="sq")
                    V.tensor_tensor(out=sq[:], in0=cv, in1=cv, op=ALU.mult)
                    rs = pDb.tile([32, 2], F32, tag="rs")
                    V.tensor_reduce(out=rs[:, 0:1], in_=cv, axis=AX.X,
                                    op=ALU.add)
                    V.tensor_reduce(out=rs[:, 1:2], in_=sq[:], axis=AX.X,
                                    op=ALU.add)
                    if w == 0:
                        V.tensor_copy(out=sc_acc[:], in_=rs[:])
                    else:
                        V.tensor_tensor(out=sc_acc[:], in0=sc_acc[:],
                                        in1=rs[:], op=ALU.add)

            cc3r = allreduce(sc_acc, [32, 2], "ar3")
            A3, B3 = bn_coefs(cc3r[:, 0:1], cc3r[:, 1:2], g_out_s[:],
                              b_out_s[:], 32, "c_")

            # ============ PHASE E: out = lrelu(bn(conv)) @ W_out + x
            with tc.tile_pool(name="pE", bufs=4) as pE:
                for w in range(NW):
                    yT = pE.tile([32, 128], F32, tag="yT")
                    nc.scalar.activation(
                        out=yT[:], in_=convT[:, w * 128:(w + 1) * 128],
                        func=AF.Identity, bias=B3[:, 0:1], scale=A3[:, 0:1])
                    leaky_dve(yT[:], yT[:], pE, "lkE", [32, 128])
                    op_ = wkp.tile([P, 128], F32, tag="wk")
                    nc.tensor.matmul(out=op_[:], lhsT=yT[:], rhs=wout_s[:],
                                     start=True, stop=True)
                    x_t = pE.tile([P, 128], F32, tag="xe")
                    nc.sync.dma_start(x_t[:], x_slice[w * P:(w + 1) * P, :])
                    o_s = pE.tile([P, 128], F32, tag="o_s")
                    nc.vector.tensor_tensor(out=o_s[:], in0=op_[:],
                                            in1=x_t[:], op=ALU.add)
                    nc.sync.dma_start(out_slice[w * P:(w + 1) * P, :],
                                      o_s[:])

    nc.compile()
    return nc


# ---------------------------------------------------------------- runner

_CACHE = {}


def get_nc(cfg: Cfg):
    if cfg not in _CACHE:
        _CACHE[cfg] = build_nc(cfg)
    return _CACHE[cfg]


def kernel(x, pos, seq, ori, edge_index, batch,
           g_in, b_in, W_in, g_mid, b_mid, Wn0, bn0, Wc,
           g_out, b_out, W_out, _run_hw=None, _cfg=None):
    x = np.asarray(x, np.float32)
    N = x.shape[0]
    cfg = _cfg or Cfg(n_real=N)
    cfg, in_maps = prep_host(
        cfg, x, np.asarray(pos, np.float32), np.asarray(seq, np.float32),
        np.asarray(ori, np.float32), np.asarray(edge_index),
        np.asarray(Wn0, np.float32), np.asarray(bn0, np.float32),
        np.asarray(W_in, np.float32), np.asarray(Wc, np.float32),
        np.asarray(W_out, np.float32),
        np.asarray(g_in, np.float32), np.asarray(b_in, np.float32),
        np.asarray(g_mid, np.float32), np.asarray(b_mid, np.float32),
        np.asarray(g_out, np.float32), np.asarray(b_out, np.float32))
    nc = get_nc(cfg)
    if _run_hw is None:
        res = bass_utils.run_bass_kernel_spmd(
            nc, in_maps, core_ids=list(range(cfg.nc)))
        outs = [res.results[c]["out_slice"] for c in range(cfg.nc)]
    else:
        outs = _run_hw(nc, in_maps, cfg)
    full = np.concatenate(outs, axis=0)[:N]
    return full.astype(np.float32)
